# revision 2
# baseline (speedup 1.0000x reference)
"""Segment softmax (GAT attention stage 4) on 8 TRN2 NeuronCores.

alpha_i = exp(e_i) / sum_{j: tgt_j == tgt_i} exp(e_j)

Node sharding (node n -> core n%8): each core owns all edges of its
12500 nodes, so there is no collective and edge_index never reaches the
device. The host packs each core's edges into a dense per-node-slot
layout: nodes are sorted by degree (descending) and grouped into chunks
of 128*w nodes (w columns per partition); chunk c has its own slot
width D_c = roundup(max degree in chunk, 8), so padding inflation is a
few percent. Pad value -60 -> exp ~ 0. Chunks are small at the head
(fast pipeline fill) and tail (short drain).

Device, per chunk: DMA in (fp16) -> ACT exp (fp16) -> DVE reduce_sum
over slots -> DVE reciprocal -> multiply by r. The multiply is spread
over THREE engines chunk-by-chunk: GPSIMD tensor_mul, DVE tensor_mul,
or per-column ACT Copy with per-partition scale=r (the scalar engine
has slack and Copy shares the exp table set). Input DMAs lead output
DMAs by LEAD chunks on the in-order Sync queue so loads are never
blocked by compute. fp16 keeps rel err ~1e-3, far under the 2e-2 gate
(segment sums >= ~0.8, so the reference's +1e-16 is negligible).
"""

import numpy as np

P = 128
NCORES = 8
NUM_NODES = 100_000
NPC_PAD = 12_544          # padded nodes/core = 128 * 98
W_LIST = (2, 5) + (7,) * 12 + (4, 2, 1)  # f-cols per chunk
NCHUNK = len(W_LIST)       # 17
#           0    1    2    3    4    5    6    7    8    9   10   11   12   13   14  15  16
MUL_ENG = "g" "g" "v" "g" "g" "a" "g" "g" "a" "g" "g" "a" "g" "v" "g" "v" "g"
LEAD = 5                   # in-DMAs lead compute by this many chunks
MUL_DELAY = 2              # multiply+store issued this many chunks after reduce
PAD_E = np.float16(-60.0)

assert sum(W_LIST) == 98 and len(MUL_ENG) == NCHUNK

_CACHE = {}


def _build(d_list):
    import concourse.mybir as mybir
    from concourse import bacc
    from concourse.tile import TileContext

    nc = bacc.Bacc(None, target_bir_lowering=False)
    tot = P * sum(w * d for w, d in zip(W_LIST, d_list))
    e_in = nc.dram_tensor("e", [tot], mybir.dt.float16, kind="ExternalInput")
    a_out = nc.dram_tensor("alpha", [tot], mybir.dt.float16, kind="ExternalOutput")

    bases = []
    b = 0
    for w, dc in zip(W_LIST, d_list):
        bases.append(b)
        b += P * w * dc

    with TileContext(nc) as tc:
        with tc.tile_pool(name="sbuf", bufs=8) as pool:
            ins, xts, rvs = {}, {}, {}

            def issue_in(c):
                w, dc = W_LIST[c], d_list[c]
                n = P * w * dc
                src = e_in[bases[c] : bases[c] + n].rearrange(
                    "(p f d) -> p f d", p=P, f=w
                )
                et = pool.tile([P, w, dc], mybir.dt.float16, tag="in", name=f"et{c}")
                nc.sync.dma_start(out=et[:], in_=src)
                ins[c] = et

            def issue_front(c):
                w, dc = W_LIST[c], d_list[c]
                et = ins.pop(c)
                xt = pool.tile([P, w, dc], mybir.dt.float16, tag="x", name=f"xt{c}")
                nc.scalar.activation(
                    xt[:], et[:], mybir.ActivationFunctionType.Exp
                )
                st = pool.tile([P, w], mybir.dt.float32, tag="s", name=f"st{c}")
                nc.vector.tensor_reduce(
                    st[:], xt[:], axis=mybir.AxisListType.X,
                    op=mybir.AluOpType.add,
                )
                rdt = (
                    mybir.dt.float32 if MUL_ENG[c] == "a" else mybir.dt.float16
                )  # ACT scale APs must be fp32
                rv = pool.tile([P, w], rdt, tag="r", name=f"rv{c}")
                with nc.allow_low_precision(reason="fp16 r; tol 2e-2"):
                    nc.vector.reciprocal(rv[:], st[:])
                xts[c], rvs[c] = xt, rv

            def issue_back(c):
                w, dc = W_LIST[c], d_list[c]
                n = P * w * dc
                dst = a_out[bases[c] : bases[c] + n].rearrange(
                    "(p f d) -> p f d", p=P, f=w
                )
                xt, rv = xts.pop(c), rvs.pop(c)
                ot = pool.tile([P, w, dc], mybir.dt.float16, tag="out", name=f"ot{c}")
                eng = MUL_ENG[c]
                if eng == "a":
                    for f in range(w):
                        nc.scalar.activation(
                            ot[:, f : f + 1, :], xt[:, f : f + 1, :],
                            mybir.ActivationFunctionType.Copy,
                            scale=rv[:, f : f + 1],
                        )
                else:
                    meng = nc.vector if eng == "v" else nc.gpsimd
                    meng.tensor_mul(
                        out=ot[:], in0=xt[:],
                        in1=rv[:, :, None].broadcast_to([P, w, dc]),
                    )
                nc.sync.dma_start(out=dst, in_=ot[:])

            for c in range(NCHUNK + LEAD + MUL_DELAY):
                if c < NCHUNK:
                    issue_in(c)
                if LEAD <= c < NCHUNK + LEAD:
                    issue_front(c - LEAD)
                if c >= LEAD + MUL_DELAY:
                    issue_back(c - LEAD - MUL_DELAY)
    nc.compile()
    return nc


def kernel(e, edge_index, num_nodes):
    from concourse.bass_utils import run_bass_kernel_spmd

    e = np.asarray(e, dtype=np.float32).ravel()
    tgt = np.asarray(edge_index)[1].astype(np.int64)
    E = tgt.shape[0]
    assert int(num_nodes) == NUM_NODES and e.shape[0] == E

    counts = np.bincount(tgt, minlength=NUM_NODES)

    # rank of each edge within its target node (any stable order)
    order = np.argsort(tgt, kind="stable")
    starts = np.concatenate(([0], np.cumsum(counts)[:-1]))
    rank = np.empty(E, dtype=np.int64)
    rank[order] = np.arange(E, dtype=np.int64) - np.repeat(starts, counts)

    # per-core degree table over padded local node ids
    core_n = tgt & 7
    ell_n = tgt >> 3
    deg = np.zeros((NCORES, NPC_PAD), dtype=np.int64)
    deg[:, : NUM_NODES // 8] = counts.reshape(-1, NCORES).T

    # sort nodes by degree desc per core; node ell -> sorted slot s
    nodesort = np.argsort(-deg, axis=1, kind="stable")
    s_of_ell = np.empty_like(nodesort)
    np.put_along_axis(s_of_ell, nodesort, np.arange(NPC_PAD)[None, :], axis=1)
    deg_sorted = np.take_along_axis(deg, nodesort, axis=1)

    # chunk slot widths: max degree within, across all cores
    nb = np.concatenate(([0], np.cumsum([P * w for w in W_LIST])))
    d_list = tuple(
        max(8, int(-8 * (-deg_sorted[:, nb[c] : nb[c + 1]].max() // 8)))
        for c in range(NCHUNK)
    )
    d_arr = np.asarray(d_list, dtype=np.int64)
    chunk_elem = P * np.asarray(W_LIST, dtype=np.int64) * d_arr
    chunk_base = np.concatenate(([0], np.cumsum(chunk_elem)[:-1]))
    tot = int(chunk_elem.sum())

    # per-edge slot: sorted node pos s -> chunk c, local node lc, rank d
    s_e = s_of_ell[core_n, ell_n]
    c_e = np.searchsorted(nb, s_e, side="right") - 1
    lc = s_e - nb[c_e]
    slot = core_n * tot + chunk_base[c_e] + lc * d_arr[c_e] + rank

    dense = np.full(NCORES * tot, PAD_E, dtype=np.float16)
    dense[slot] = e.astype(np.float16)
    dense = dense.reshape(NCORES, tot)

    if d_list not in _CACHE:
        _CACHE[d_list] = _build(d_list)
    nc = _CACHE[d_list]

    in_maps = [{"e": dense[c]} for c in range(NCORES)]
    res = run_bass_kernel_spmd(nc, in_maps, core_ids=list(range(NCORES)))

    out = np.stack([res.results[c]["alpha"] for c in range(NCORES)])
    return out.reshape(-1)[slot].astype(np.float32)


# revision 3
# speedup vs baseline: 1.0834x; 1.0834x over previous
"""Segment softmax (GAT attention stage 4) on 8 TRN2 NeuronCores.

alpha_i = exp(e_i) / sum_{j: tgt_j == tgt_i} exp(e_j)

Node sharding (node n -> core n%8): each core owns all edges of its
12500 nodes, so there is no collective and edge_index never reaches the
device. The host packs each core's edges into a dense per-node-slot
layout: nodes are sorted by degree (descending) and grouped into chunks
of 128*w nodes (w columns per partition); chunk c has its own slot
width D_c = roundup(max degree in chunk, 8), so padding inflation is a
few percent. Pad value -60 -> exp ~ 0. Chunks are small at the head
(fast pipeline fill) and tail (short drain).

Device, per chunk: DMA in (fp16) -> ACT exp (fp16) -> DVE reduce_sum
over slots -> DVE reciprocal -> multiply by r. The multiply is spread
over THREE engines chunk-by-chunk: GPSIMD tensor_mul, DVE tensor_mul,
or per-column ACT Copy with per-partition scale=r (the scalar engine
has slack and Copy shares the exp table set). Input DMAs lead output
DMAs by LEAD chunks on the in-order Sync queue so loads are never
blocked by compute. fp16 keeps rel err ~1e-3, far under the 2e-2 gate
(segment sums >= ~0.8, so the reference's +1e-16 is negligible).
"""

import numpy as np

P = 128
NCORES = 8
NUM_NODES = 100_000
NPC_PAD = 12_544          # padded nodes/core = 128 * 98
W_LIST = (2, 5) + (7,) * 12 + (4, 2, 1)  # f-cols per chunk
NCHUNK = len(W_LIST)       # 17
#           0    1    2    3    4    5    6    7    8    9   10   11   12   13   14  15  16
MUL_ENG = "g" "g" "v" "g" "g" "a" "g" "g" "a" "g" "g" "a" "g" "v" "g" "g" "v"
LEAD = 5                   # in-DMAs lead compute by this many chunks
MUL_DELAY = 3              # multiply+store issued this many chunks after reduce
PAD_E = np.float16(-60.0)

assert sum(W_LIST) == 98 and len(MUL_ENG) == NCHUNK

_CACHE = {}


def _build(d_list):
    import concourse.mybir as mybir
    from concourse import bacc
    from concourse.tile import TileContext

    nc = bacc.Bacc(None, target_bir_lowering=False)
    tot = P * sum(w * d for w, d in zip(W_LIST, d_list))
    e_in = nc.dram_tensor("e", [tot], mybir.dt.float16, kind="ExternalInput")
    a_out = nc.dram_tensor("alpha", [tot], mybir.dt.float16, kind="ExternalOutput")

    bases = []
    b = 0
    for w, dc in zip(W_LIST, d_list):
        bases.append(b)
        b += P * w * dc

    with TileContext(nc) as tc:
        with tc.tile_pool(name="sbuf", bufs=8) as pool:
            ins, xts, rvs = {}, {}, {}

            def issue_in(c):
                w, dc = W_LIST[c], d_list[c]
                n = P * w * dc
                src = e_in[bases[c] : bases[c] + n].rearrange(
                    "(p f d) -> p f d", p=P, f=w
                )
                et = pool.tile([P, w, dc], mybir.dt.float16, tag="in", name=f"et{c}")
                nc.sync.dma_start(out=et[:], in_=src)
                ins[c] = et

            def issue_front(c):
                w, dc = W_LIST[c], d_list[c]
                et = ins.pop(c)
                xt = pool.tile([P, w, dc], mybir.dt.float16, tag="x", name=f"xt{c}")
                nc.scalar.activation(
                    xt[:], et[:], mybir.ActivationFunctionType.Exp
                )
                st = pool.tile([P, w], mybir.dt.float32, tag="s", name=f"st{c}")
                nc.vector.tensor_reduce(
                    st[:], xt[:], axis=mybir.AxisListType.X,
                    op=mybir.AluOpType.add,
                )
                rdt = (
                    mybir.dt.float32 if MUL_ENG[c] == "a" else mybir.dt.float16
                )  # ACT scale APs must be fp32
                rv = pool.tile([P, w], rdt, tag="r", name=f"rv{c}")
                with nc.allow_low_precision(reason="fp16 r; tol 2e-2"):
                    nc.vector.reciprocal(rv[:], st[:])
                xts[c], rvs[c] = xt, rv

            def issue_back(c):
                w, dc = W_LIST[c], d_list[c]
                n = P * w * dc
                dst = a_out[bases[c] : bases[c] + n].rearrange(
                    "(p f d) -> p f d", p=P, f=w
                )
                xt, rv = xts.pop(c), rvs.pop(c)
                ot = pool.tile([P, w, dc], mybir.dt.float16, tag="out", name=f"ot{c}")
                eng = MUL_ENG[c]
                if eng == "a":
                    for f in range(w):
                        nc.scalar.activation(
                            ot[:, f : f + 1, :], xt[:, f : f + 1, :],
                            mybir.ActivationFunctionType.Copy,
                            scale=rv[:, f : f + 1],
                        )
                else:
                    meng = nc.vector if eng == "v" else nc.gpsimd
                    meng.tensor_mul(
                        out=ot[:], in0=xt[:],
                        in1=rv[:, :, None].broadcast_to([P, w, dc]),
                    )
                nc.sync.dma_start(out=dst, in_=ot[:])

            for c in range(NCHUNK + LEAD + MUL_DELAY):
                if c < NCHUNK:
                    issue_in(c)
                if LEAD <= c < NCHUNK + LEAD:
                    issue_front(c - LEAD)
                if c >= LEAD + MUL_DELAY:
                    issue_back(c - LEAD - MUL_DELAY)
    nc.compile()
    return nc


def kernel(e, edge_index, num_nodes):
    from concourse.bass_utils import run_bass_kernel_spmd

    e = np.asarray(e, dtype=np.float32).ravel()
    tgt = np.asarray(edge_index)[1].astype(np.int64)
    E = tgt.shape[0]
    assert int(num_nodes) == NUM_NODES and e.shape[0] == E

    counts = np.bincount(tgt, minlength=NUM_NODES)

    # rank of each edge within its target node (any stable order)
    order = np.argsort(tgt, kind="stable")
    starts = np.concatenate(([0], np.cumsum(counts)[:-1]))
    rank = np.empty(E, dtype=np.int64)
    rank[order] = np.arange(E, dtype=np.int64) - np.repeat(starts, counts)

    # per-core degree table over padded local node ids
    core_n = tgt & 7
    ell_n = tgt >> 3
    deg = np.zeros((NCORES, NPC_PAD), dtype=np.int64)
    deg[:, : NUM_NODES // 8] = counts.reshape(-1, NCORES).T

    # sort nodes by degree desc per core; node ell -> sorted slot s
    nodesort = np.argsort(-deg, axis=1, kind="stable")
    s_of_ell = np.empty_like(nodesort)
    np.put_along_axis(s_of_ell, nodesort, np.arange(NPC_PAD)[None, :], axis=1)
    deg_sorted = np.take_along_axis(deg, nodesort, axis=1)

    # chunk slot widths: max degree within, across all cores
    nb = np.concatenate(([0], np.cumsum([P * w for w in W_LIST])))
    d_list = tuple(
        max(8, int(-8 * (-deg_sorted[:, nb[c] : nb[c + 1]].max() // 8)))
        for c in range(NCHUNK)
    )
    d_arr = np.asarray(d_list, dtype=np.int64)
    chunk_elem = P * np.asarray(W_LIST, dtype=np.int64) * d_arr
    chunk_base = np.concatenate(([0], np.cumsum(chunk_elem)[:-1]))
    tot = int(chunk_elem.sum())

    # per-edge slot: sorted node pos s -> chunk c, local node lc, rank d
    s_e = s_of_ell[core_n, ell_n]
    c_e = np.searchsorted(nb, s_e, side="right") - 1
    lc = s_e - nb[c_e]
    slot = core_n * tot + chunk_base[c_e] + lc * d_arr[c_e] + rank

    dense = np.full(NCORES * tot, PAD_E, dtype=np.float16)
    dense[slot] = e.astype(np.float16)
    dense = dense.reshape(NCORES, tot)

    if d_list not in _CACHE:
        _CACHE[d_list] = _build(d_list)
    nc = _CACHE[d_list]

    in_maps = [{"e": dense[c]} for c in range(NCORES)]
    res = run_bass_kernel_spmd(nc, in_maps, core_ids=list(range(NCORES)))

    out = np.stack([res.results[c]["alpha"] for c in range(NCORES)])
    return out.reshape(-1)[slot].astype(np.float32)


# revision 4
# speedup vs baseline: 1.0899x; 1.0060x over previous
"""Segment softmax (GAT attention stage 4) on 8 TRN2 NeuronCores.

alpha_i = exp(e_i) / sum_{j: tgt_j == tgt_i} exp(e_j)

Node sharding (node n -> core n%8): each core owns all edges of its
12500 nodes, so there is no collective and edge_index never reaches the
device. The host packs each core's edges into a dense per-node-slot
layout: nodes are sorted by degree (descending) and grouped into chunks
of 128*w nodes (w columns per partition); chunk c has its own slot
width D_c = roundup(max degree in chunk, 8), so padding inflation is a
few percent. Pad value -60 -> exp ~ 0. Chunks are small at the head
(fast pipeline fill) and tail (short drain).

Device, per chunk: DMA in (fp16) -> ACT exp (fp16) -> DVE reduce_sum
over slots -> DVE reciprocal -> multiply by r. The multiply is spread
over THREE engines chunk-by-chunk: GPSIMD tensor_mul, DVE tensor_mul,
or per-column ACT Copy with per-partition scale=r (the scalar engine
has slack and Copy shares the exp table set). Input DMAs lead output
DMAs by LEAD chunks on the in-order Sync queue so loads are never
blocked by compute. fp16 keeps rel err ~1e-3, far under the 2e-2 gate
(segment sums >= ~0.8, so the reference's +1e-16 is negligible).
"""

import numpy as np

P = 128
NCORES = 8
NUM_NODES = 100_000
NPC_PAD = 12_544          # padded nodes/core = 128 * 98
W_LIST = (2, 5) + (7,) * 12 + (4, 2, 1)  # f-cols per chunk
NCHUNK = len(W_LIST)       # 17
#           0    1    2    3    4    5    6    7    8    9   10   11   12   13   14  15  16
MUL_ENG = "g" "g" "v" "g" "g" "a" "g" "g" "a" "g" "g" "a" "g" "v" "g" "g" "v"
LEAD = 5                   # in-DMAs lead compute by this many chunks
MUL_DELAY = 5              # multiply+store issued this many chunks after reduce
PAD_E = np.float16(-60.0)

assert sum(W_LIST) == 98 and len(MUL_ENG) == NCHUNK

_CACHE = {}


def _build(d_list):
    import concourse.mybir as mybir
    from concourse import bacc
    from concourse.tile import TileContext

    nc = bacc.Bacc(None, target_bir_lowering=False)
    tot = P * sum(w * d for w, d in zip(W_LIST, d_list))
    e_in = nc.dram_tensor("e", [tot], mybir.dt.float16, kind="ExternalInput")
    a_out = nc.dram_tensor("alpha", [tot], mybir.dt.float16, kind="ExternalOutput")

    bases = []
    b = 0
    for w, dc in zip(W_LIST, d_list):
        bases.append(b)
        b += P * w * dc

    with TileContext(nc) as tc:
        with tc.tile_pool(name="sbuf", bufs=8) as pool:
            ins, xts, rvs = {}, {}, {}

            def issue_in(c):
                w, dc = W_LIST[c], d_list[c]
                n = P * w * dc
                src = e_in[bases[c] : bases[c] + n].rearrange(
                    "(p f d) -> p f d", p=P, f=w
                )
                et = pool.tile([P, w, dc], mybir.dt.float16, tag="in", name=f"et{c}")
                nc.sync.dma_start(out=et[:], in_=src)
                ins[c] = et

            def issue_front(c):
                w, dc = W_LIST[c], d_list[c]
                et = ins.pop(c)
                xt = pool.tile([P, w, dc], mybir.dt.float16, tag="x", name=f"xt{c}")
                nc.scalar.activation(
                    xt[:], et[:], mybir.ActivationFunctionType.Exp
                )
                st = pool.tile([P, w], mybir.dt.float32, tag="s", name=f"st{c}")
                nc.vector.tensor_reduce(
                    st[:], xt[:], axis=mybir.AxisListType.X,
                    op=mybir.AluOpType.add,
                )
                rdt = (
                    mybir.dt.float32 if MUL_ENG[c] == "a" else mybir.dt.float16
                )  # ACT scale APs must be fp32
                rv = pool.tile([P, w], rdt, tag="r", name=f"rv{c}")
                with nc.allow_low_precision(reason="fp16 r; tol 2e-2"):
                    nc.vector.reciprocal(rv[:], st[:])
                xts[c], rvs[c] = xt, rv

            def issue_back(c):
                w, dc = W_LIST[c], d_list[c]
                n = P * w * dc
                dst = a_out[bases[c] : bases[c] + n].rearrange(
                    "(p f d) -> p f d", p=P, f=w
                )
                xt, rv = xts.pop(c), rvs.pop(c)
                ot = pool.tile([P, w, dc], mybir.dt.float16, tag="out", name=f"ot{c}")
                eng = MUL_ENG[c]
                if eng == "a":
                    for f in range(w):
                        nc.scalar.activation(
                            ot[:, f : f + 1, :], xt[:, f : f + 1, :],
                            mybir.ActivationFunctionType.Copy,
                            scale=rv[:, f : f + 1],
                        )
                else:
                    meng = nc.vector if eng == "v" else nc.gpsimd
                    meng.tensor_mul(
                        out=ot[:], in0=xt[:],
                        in1=rv[:, :, None].broadcast_to([P, w, dc]),
                    )
                nc.sync.dma_start(out=dst, in_=ot[:])

            for c in range(NCHUNK + LEAD + MUL_DELAY):
                if c < NCHUNK:
                    issue_in(c)
                if LEAD <= c < NCHUNK + LEAD:
                    issue_front(c - LEAD)
                if c >= LEAD + MUL_DELAY:
                    issue_back(c - LEAD - MUL_DELAY)
    nc.compile()
    return nc


def kernel(e, edge_index, num_nodes):
    from concourse.bass_utils import run_bass_kernel_spmd

    e = np.asarray(e, dtype=np.float32).ravel()
    tgt = np.asarray(edge_index)[1].astype(np.int64)
    E = tgt.shape[0]
    assert int(num_nodes) == NUM_NODES and e.shape[0] == E

    counts = np.bincount(tgt, minlength=NUM_NODES)

    # rank of each edge within its target node (any stable order)
    order = np.argsort(tgt, kind="stable")
    starts = np.concatenate(([0], np.cumsum(counts)[:-1]))
    rank = np.empty(E, dtype=np.int64)
    rank[order] = np.arange(E, dtype=np.int64) - np.repeat(starts, counts)

    # per-core degree table over padded local node ids
    core_n = tgt & 7
    ell_n = tgt >> 3
    deg = np.zeros((NCORES, NPC_PAD), dtype=np.int64)
    deg[:, : NUM_NODES // 8] = counts.reshape(-1, NCORES).T

    # sort nodes by degree desc per core; node ell -> sorted slot s
    nodesort = np.argsort(-deg, axis=1, kind="stable")
    s_of_ell = np.empty_like(nodesort)
    np.put_along_axis(s_of_ell, nodesort, np.arange(NPC_PAD)[None, :], axis=1)
    deg_sorted = np.take_along_axis(deg, nodesort, axis=1)

    # chunk slot widths: max degree within, across all cores
    nb = np.concatenate(([0], np.cumsum([P * w for w in W_LIST])))
    d_list = tuple(
        max(8, int(-8 * (-deg_sorted[:, nb[c] : nb[c + 1]].max() // 8)))
        for c in range(NCHUNK)
    )
    d_arr = np.asarray(d_list, dtype=np.int64)
    chunk_elem = P * np.asarray(W_LIST, dtype=np.int64) * d_arr
    chunk_base = np.concatenate(([0], np.cumsum(chunk_elem)[:-1]))
    tot = int(chunk_elem.sum())

    # per-edge slot: sorted node pos s -> chunk c, local node lc, rank d
    s_e = s_of_ell[core_n, ell_n]
    c_e = np.searchsorted(nb, s_e, side="right") - 1
    lc = s_e - nb[c_e]
    slot = core_n * tot + chunk_base[c_e] + lc * d_arr[c_e] + rank

    dense = np.full(NCORES * tot, PAD_E, dtype=np.float16)
    dense[slot] = e.astype(np.float16)
    dense = dense.reshape(NCORES, tot)

    if d_list not in _CACHE:
        _CACHE[d_list] = _build(d_list)
    nc = _CACHE[d_list]

    in_maps = [{"e": dense[c]} for c in range(NCORES)]
    res = run_bass_kernel_spmd(nc, in_maps, core_ids=list(range(NCORES)))

    out = np.stack([res.results[c]["alpha"] for c in range(NCORES)])
    return out.reshape(-1)[slot].astype(np.float32)
